# revision 24
# baseline (speedup 1.0000x reference)
"""Trainium2 Bass kernel for DiscreteResidualQuantization.

  z_q = z_e + RMSNormLinear(h - z_e),  z_e = softmax(-||h-c||^2) @ C

Sharding: data-parallel over the N=8192 tokens across 8 NeuronCores
(1024 tokens/core); codebook and MLP params replicated.

Per-core algorithm (T=1024 tokens, D=1024, K=8192 codebook entries),
processed in G=2 groups of TG=512 tokens:

  pass1: S^T[k_chunk] = 2*(C @ h^T) - |c|^2 in ONE fp16 matmul pass
         (fp16 keeps ~11 mantissa bits: logit abs err ~0.1, which the
         softmax tolerates; h_sq cancels in softmax so it is never
         computed) + running elementwise row-max across chunks
  max:   cross-partition max via PE-transpose + free-dim reduce,
         broadcast back via ones-matmul
  pass2: P^T = exp(S^T - m) written fp16 in place (ACT) into the low
         half of each fp32 S^T tile
  GEMM2: z_e_unnorm += P^T.T @ C  (both operands fp16, P^T tiles are
         directly the stationary operand); l = ones^T @ P^T accumulates
         on the PE in a parallel PSUM tile during the dh=0 sweep
  MLP:   r = h - z_e/l; x = r/(rms+eps); z_q = z_e/l + x @ (W*scale)^T + b
         (scale folded into W on host, bias via rank-1 ones-matmul,
         x^T/W^T fp16)

kernel(**inputs) is self-contained: host-side it only reshapes/transposes/
casts inputs, then runs one SPMD NEFF on cores 0-7 and concatenates.
"""

import numpy as np

import concourse.bacc as bacc
import concourse.mybir as mybir
import concourse.tile as tile
from concourse.bass_utils import run_bass_kernel_spmd
from concourse.masks import make_identity

F32 = mybir.dt.float32
F32R = mybir.dt.float32r
F16 = mybir.dt.float16
AF = mybir.ActivationFunctionType
ALU = mybir.AluOpType
AX = mybir.AxisListType

N_CORES = 8
EPS = 1e-8


def build_nc(T=1024, D=1024, K=8192, TG=512, stop_after=None, reps=1, ablate=()):
    """Build + compile the per-core program. T tokens/core, contraction dim D,
    K codebook entries, TG tokens per group. reps repeats the whole
    computation in-NEFF (timing); stop_after/ablate are phase/feature
    cutoffs for microbenchmarks."""
    ablate = frozenset(ablate)
    G = T // TG          # token groups
    TS = TG // 128       # 128-token subtiles per group
    DC = D // 128        # contraction chunks
    KC = K // 128        # codebook chunks
    DH = D // 512        # output D halves (512-wide psum tiles)
    assert T % TG == 0 and TG % 128 == 0 and D % 512 == 0 and K % 128 == 0

    nc = bacc.Bacc("TRN2", target_bir_lowering=False, debug=False,
                   num_devices=N_CORES)

    # cbt16 is host-pretiled so each k-chunk DMA has contiguous 2KB
    # partition lines.
    hT16 = nc.dram_tensor("hT16", [D, T], F16, kind="ExternalInput").ap()
    h_nat = nc.dram_tensor("h_nat", [T, D], F32, kind="ExternalInput").ap()
    cbt16 = nc.dram_tensor("cbt16", [KC, 128, DC, 128], F16,
                           kind="ExternalInput").ap()
    cb16 = nc.dram_tensor("cb16", [K, D], F16, kind="ExternalInput").ap()
    csqn = nc.dram_tensor("csqn", [128, KC], F32, kind="ExternalInput").ap()
    wT16 = nc.dram_tensor("wT16", [D, D], F16, kind="ExternalInput").ap()
    b_row = nc.dram_tensor("b_row", [1, D], F16, kind="ExternalInput").ap()
    ones_row = nc.dram_tensor("ones_row", [1, 128], F32R, kind="ExternalInput").ap()
    ones16_row = nc.dram_tensor("ones16_row", [1, 128], F16, kind="ExternalInput").ap()
    ones16_col = nc.dram_tensor("ones16_col", [128, 1], F16, kind="ExternalInput").ap()
    zq = nc.dram_tensor("zq", [T, D], F32, kind="ExternalOutput").ap()

    wT_tiled = wT16.rearrange("(ic ip) o -> ip ic o", ip=128)
    hT_tiled = hT16.rearrange("(dc dp) t -> dp dc t", dp=128)
    cbt_paired = cbt16.rearrange("(kp kk) p dc x -> kp p kk dc x", kk=2)

    with tile.TileContext(nc) as tc:
        with (
            tc.tile_pool(name="singles", bufs=1) as singles,
            tc.tile_pool(name="wlep", bufs=DC * DH) as wlep,
            tc.tile_pool(name="stp", bufs=KC) as stp,
            tc.tile_pool(name="htp", bufs=1) as htp,
            tc.tile_pool(name="cbtp", bufs=2) as cbtp,
            tc.tile_pool(name="cbp", bufs=2) as cbp,
            tc.tile_pool(name="zep", bufs=TS * DH) as zep,
            tc.tile_pool(name="hp", bufs=2) as hp,
            tc.tile_pool(name="xp", bufs=4) as xp,
            tc.tile_pool(name="xtp", bufs=1) as xtp,
            tc.tile_pool(name="tmpp", bufs=2) as tmpp,
            tc.tile_pool(name="mp", bufs=1) as mp,
            tc.tile_pool(name="smalls", bufs=8) as smalls,
            tc.tile_pool(name="ps_mm", bufs=3, space="PSUM") as ps_mm,
            tc.tile_pool(name="ps_acc", bufs=TS, space="PSUM") as ps_acc,
            tc.tile_pool(name="ps_sm", bufs=1, space="PSUM") as ps_sm,
        ):
            # ---- constants resident in SBUF ----
            ident_f32 = singles.tile([128, 128], F32)
            make_identity(nc, ident_f32)
            ident_f16 = singles.tile([128, 128], F16)
            make_identity(nc, ident_f16)
            ones_f32 = singles.tile([1, 128], F32R)
            nc.sync.dma_start(out=ones_f32, in_=ones_row)
            csqn_sb = singles.tile([128, KC], F32)
            nc.sync.dma_start(out=csqn_sb, in_=csqn)
            b_sb = singles.tile([1, D], F16)
            nc.sync.dma_start(out=b_sb, in_=b_row)
            ones_f16 = singles.tile([1, 128], F16)
            nc.sync.dma_start(out=ones_f16, in_=ones16_row)
            ones_col16 = singles.tile([128, 1], F16)
            nc.sync.dma_start(out=ones_col16, in_=ones16_col)
            # W^T loaded once, fp16
            wle = [
                [wlep.tile([128, 512], F16, name="wle") for _ in range(DH)]
                for _ in range(DC)
            ]
            for ic in range(DC):
                for dh in range(DH):
                    nc.sync.dma_start(
                        out=wle[ic][dh],
                        in_=wT_tiled[:, ic, dh * 512:(dh + 1) * 512],
                    )

            cbT_static = None
            cb_static = None
            if "p1_nodma" in ablate:
                cbT_static = singles.tile([128, 2, DC, 128], F16)
                nc.sync.dma_start(out=cbT_static, in_=cbt_paired[0])
            if "g2_static_cb" in ablate:
                cb_static = cbp.tile([128, 512], F16, name="cb_static")
                nc.scalar.dma_start(out=cb_static, in_=cb16[0:128, 0:512])
            for g in range(reps * (G if stop_after != "null" else 0)):
                g = g % G
                gtok = g * TG
                # ================= pass 1: S^T + running row-max ============
                hT_sb = htp.tile([128, DC, TG], F16, name="hT_sb")
                nc.sync.dma_start(
                    out=hT_sb, in_=hT_tiled[:, :, gtok:gtok + TG]
                )
                rowmax = mp.tile([128, TG], F32, name="rowmax")
                nc.vector.memset(rowmax, -3.0e38)

                st_tiles = []
                for k in range(KC):
                    if "p1_nodma" in ablate:
                        cbT_sb = cbT_static[:, 0]
                    else:
                        cbT_sb = cbtp.tile([128, DC, 128], F16, name="cbT_sb")
                        nc.sync.dma_start(out=cbT_sb, in_=cbt16[k])
                    ps = ps_mm.tile([128, TG], F32, name="g1ps")
                    for dc in range(DC):
                        nc.tensor.matmul(
                            ps,
                            cbT_sb[:, dc, :],
                            hT_sb[:, dc, :],
                            start=(dc == 0),
                            stop=(dc == DC - 1),
                        )
                    if "p1_bare" in ablate:
                        continue
                    stk = stp.tile([128, TG], F32, name="stk", tag="st")
                    if "drain_pool" in ablate:
                        nc.gpsimd.tensor_scalar(
                            out=stk, in0=ps, scalar1=2.0,
                            scalar2=csqn_sb[:, k:k + 1],
                            op0=ALU.mult, op1=ALU.add,
                        )
                    elif "drain_dve" in ablate:
                        nc.vector.tensor_scalar(
                            out=stk, in0=ps, scalar1=2.0,
                            scalar2=csqn_sb[:, k:k + 1],
                            op0=ALU.mult, op1=ALU.add,
                        )
                    else:
                        nc.scalar.activation(
                            out=stk, in_=ps, func=AF.Identity,
                            bias=csqn_sb[:, k:k + 1], scale=2.0,
                        )
                    nc.vector.tensor_max(rowmax, rowmax, stk)
                    st_tiles.append(stk)

                if stop_after == "pass1":
                    continue
                # ============ cross-partition max -> m_bcast ================
                m_cols = []
                for j in range(TS):
                    pst = ps_sm.tile([128, 128], F32, name="pst", tag="pssm")
                    nc.tensor.transpose(
                        pst, rowmax[:, j * 128:(j + 1) * 128], ident_f32
                    )
                    mcol = smalls.tile([128, 1], F32, name="mcol")
                    nc.vector.tensor_reduce(
                        out=mcol, in_=pst, axis=AX.X, op=ALU.max
                    )
                    m_cols.append(mcol)
                m_row = mp.tile([1, TG], F32R, name="m_row")
                for j in range(TS):
                    pst2 = ps_sm.tile([1, 128], F32, name="pst2", tag="pssm")
                    nc.tensor.transpose(pst2, m_cols[j], ident_f32)
                    nc.vector.tensor_copy(m_row[:, j * 128:(j + 1) * 128], pst2)
                ps_b = ps_sm.tile([128, TG], F32, name="ps_b", tag="pssm")
                nc.tensor.matmul(
                    ps_b, ones_f32, m_row,
                    start=True, stop=True,
                )
                m_b = mp.tile([128, TG], F32, name="m_b")
                nc.vector.tensor_copy(m_b, ps_b)

                if stop_after == "mmach":
                    continue
                # ====== pass 2: P^T = exp(S^T - m) fp16 in place ============
                pt_tiles = []
                for k in range(KC):
                    tmp = tmpp.tile([128, TG], F32, name="tmp")
                    nc.vector.tensor_sub(tmp, st_tiles[k], m_b)
                    ptk = st_tiles[k].bitcast(F16)[:, 0:TG]
                    nc.scalar.activation(out=ptk, in_=tmp, func=AF.Exp)
                    pt_tiles.append(ptk)

                # l = ones^T @ P^T accumulates on the PE inside GEMM2 dh=0
                l_ps = ps_sm.tile([1, TG], F32, name="l_ps", tag="pssm")

                if stop_after == "pass2":
                    for k in range(KC):
                        nc.tensor.matmul(
                            l_ps, ones_col16, pt_tiles[k],
                            start=(k == 0), stop=(k == KC - 1),
                        )
                    l_rowd = mp.tile([1, TG], F32, name="m_row")
                    nc.vector.tensor_copy(l_rowd, l_ps)
                    nc.sync.dma_start(out=zq[0:1, 0:TG], in_=l_rowd)
                    continue
                # ==================== GEMM2: z_e unnormalized ===============
                zeun = [
                    [
                        zep.tile([128, 512], F32, name="zeun", tag="ze")
                        for _ in range(DH)
                    ]
                    for _ in range(TS)
                ]
                recs, rec_negs = [], []
                for dh in range(DH):
                    accs = [
                        ps_acc.tile([128, 512], F32, name="acc", tag="acc")
                        for _ in range(TS)
                    ]
                    for k in range(KC):
                        if "g2_static_cb" in ablate:
                            cb_sb = cb_static
                        else:
                            cb_sb = cbp.tile([128, 512], F16, name="cb_sb")
                            nc.scalar.dma_start(
                                out=cb_sb,
                                in_=cb16[k * 128:(k + 1) * 128,
                                         dh * 512:(dh + 1) * 512],
                            )
                        for ts in range(TS):
                            nc.tensor.matmul(
                                accs[ts],
                                pt_tiles[k][:, ts * 128:(ts + 1) * 128],
                                cb_sb,
                                start=(k == 0),
                                stop=(k == KC - 1),
                            )
                        if dh == 0:
                            nc.tensor.matmul(
                                l_ps, ones_col16, pt_tiles[k],
                                start=(k == 0), stop=(k == KC - 1),
                            )
                    if dh == 0:
                        l_row = mp.tile([1, TG], F32, name="m_row")
                        nc.vector.tensor_copy(l_row, l_ps)
                        for j in range(TS):
                            pst3 = ps_sm.tile([128, 1], F32, name="pst3",
                                              tag="pssm")
                            nc.tensor.transpose(
                                pst3,
                                l_row[:, j * 128:(j + 1) * 128],
                                ident_f32[0:1, 0:1],
                            )
                            rec = smalls.tile([128, 1], F32, name="rec")
                            nc.vector.reciprocal(rec, pst3)
                            rec_neg = smalls.tile([128, 1], F32,
                                                  name="rec_neg")
                            nc.vector.tensor_scalar_mul(rec_neg, rec, -1.0)
                            recs.append(rec)
                            rec_negs.append(rec_neg)
                    for ts in range(TS):
                        nc.vector.tensor_copy(zeun[ts][dh], accs[ts])

                if stop_after == "gemm2":
                    continue
                # ==================== MLP + output ==========================
                # Loop A (DVE/ACT only): r, rms, x for ALL token tiles —
                # emitted before any MLP PE work so the PE FIFO never
                # head-of-line blocks on a per-tile rinv chain.
                x_tiles = []
                for ts in range(TS):
                    trow = gtok + ts * 128
                    h_sb = hp.tile([128, D], F32, name="h_sb")
                    nc.scalar.dma_start(out=h_sb, in_=h_nat[trow:trow + 128, :])
                    # r = h - z_e/l  (in place over h_sb)
                    for dh in range(DH):
                        sl = slice(dh * 512, (dh + 1) * 512)
                        nc.vector.scalar_tensor_tensor(
                            out=h_sb[:, sl], in0=zeun[ts][dh],
                            scalar=rec_negs[ts], in1=h_sb[:, sl],
                            op0=ALU.mult, op1=ALU.add,
                        )
                    # rms: r^2 and its row-sum in ONE ACT op (accum_out)
                    x_sb = xp.tile([128, D], F16, name="x_sb")
                    rsum = smalls.tile([128, 1], F32, name="rsum")
                    nc.scalar.activation(
                        out=x_sb, in_=h_sb, func=AF.Square, accum_out=rsum
                    )
                    rms = smalls.tile([128, 1], F32, name="rms")
                    nc.scalar.activation(
                        out=rms, in_=rsum, func=AF.Sqrt, scale=1.0 / D
                    )
                    nc.vector.tensor_scalar_add(rms, rms, EPS)
                    rinv = smalls.tile([128, 1], F32, name="rinv")
                    nc.vector.reciprocal(rinv, rms)
                    # x = r * rinv on ACT (per-partition vector scale)
                    nc.scalar.activation(
                        out=x_sb, in_=h_sb, func=AF.Copy, scale=rinv
                    )
                    x_tiles.append(x_sb)
                # Loop B (PE-heavy): transposes + matmuls + combine + store
                for ts in range(TS):
                    trow = gtok + ts * 128
                    x_sb = x_tiles[ts]
                    # x^T via PE transposes, fp16 PSUM->SBUF copies
                    xT_sb = xtp.tile([128, DC, 128], F16, name="xT_sb")
                    for ic in range(DC):
                        pstx = ps_acc.tile(
                            [128, 128], F16, name="pstx", tag="acc"
                        )
                        nc.tensor.transpose(
                            pstx, x_sb[:, ic * 128:(ic + 1) * 128], ident_f16
                        )
                        nc.vector.tensor_copy(xT_sb[:, ic, :], pstx)
                    # z_mlp = x @ (W*scale)^T + b ; z_q = z_e/l + z_mlp
                    for dh in range(DH):
                        psm = ps_acc.tile([128, 512], F32, name="mlpps",
                                          tag="acc")
                        for ic in range(DC):
                            nc.tensor.matmul(
                                psm,
                                xT_sb[:, ic, :],
                                wle[ic][dh],
                                start=(ic == 0),
                                stop=False,
                            )
                        nc.tensor.matmul(
                            psm,
                            ones_f16,
                            b_sb[:, dh * 512:(dh + 1) * 512],
                            start=False, stop=True,
                        )
                        nc.vector.scalar_tensor_tensor(
                            out=zeun[ts][dh], in0=zeun[ts][dh],
                            scalar=recs[ts], in1=psm,
                            op0=ALU.mult, op1=ALU.add,
                        )
                        nc.scalar.dma_start(
                            out=zq[trow:trow + 128, dh * 512:(dh + 1) * 512],
                            in_=zeun[ts][dh],
                        )

    nc.compile()
    return nc


def prep_inputs(h, codebook, scale, W, b, n_cores=N_CORES):
    """Host-side reshapes/transposes/casts -> per-core in_maps."""
    h = np.asarray(h, dtype=np.float32)
    codebook = np.ascontiguousarray(np.asarray(codebook, dtype=np.float32))
    scale = np.asarray(scale, dtype=np.float32)
    W = np.asarray(W, dtype=np.float32)
    b = np.asarray(b, dtype=np.float32)

    K, D = codebook.shape
    T = h.shape[0] // n_cores

    KC, DC = K // 128, D // 128
    cbT16 = np.ascontiguousarray(codebook.T).astype(np.float16)  # [D, K]
    # [D, K] -> [KC, 128dp, DC, 128k] contiguous
    cbt16 = np.ascontiguousarray(
        cbT16.reshape(DC, 128, KC, 128).transpose(2, 1, 0, 3)
    )
    cb16 = codebook.astype(np.float16)
    csqn = (-np.sum(codebook.astype(np.float64) ** 2, axis=1)).astype(np.float32)
    csqn2d = np.ascontiguousarray(csqn.reshape(K // 128, 128).T)
    wT16 = np.ascontiguousarray((W * scale[None, :]).T).astype(np.float16)
    b_row = np.ascontiguousarray(b.reshape(1, D)).astype(np.float16)

    in_maps = []
    for c in range(n_cores):
        hc = np.ascontiguousarray(h[c * T:(c + 1) * T])
        hcT16 = np.ascontiguousarray(hc.T).astype(np.float16)
        in_maps.append({
            "ones_row": np.ones((1, 128), dtype=np.float32),
            "ones16_row": np.ones((1, 128), dtype=np.float16),
            "ones16_col": np.ones((128, 1), dtype=np.float16),
            "hT16": hcT16,
            "h_nat": hc,
            "cbt16": cbt16,
            "cb16": cb16,
            "csqn": csqn2d,
            "wT16": wT16,
            "b_row": b_row,
        })
    return in_maps


_NC_CACHE = {}


def get_nc():
    if "nc" not in _NC_CACHE:
        _NC_CACHE["nc"] = build_nc()
    return _NC_CACHE["nc"]


def kernel(h, codebook, scale, W, b):
    nc = get_nc()
    in_maps = prep_inputs(h, codebook, scale, W, b)
    res = run_bass_kernel_spmd(nc, in_maps, core_ids=list(range(N_CORES)))
    out = np.concatenate([r["zq"] for r in res.results], axis=0)
    return out.astype(np.float32)
